# revision 35
# baseline (speedup 1.0000x reference)
"""Trainium2 Bass kernel: batched dense GAT (PyG GATConv, eval, concat heads).

Contract: kernel(**inputs) takes FULL inputs (numpy/jax arrays) and returns the
FULL output [B, N, H*C] float32. Internally shards across 8 NeuronCores:
core c handles graph b = c//2, target-node half j0 = (c%2)*1024.  The host
reorders each core's node axis so its NJ target nodes come first (fT columns
and adj rows permuted to match), which lets a_dst fall out of the same
projection pass as a_src.

Math (per graph):
  x = f @ W                       [N, H, C]
  a_src[i,h] = <x[i,h,:], att_src[h,:]>,  a_dst likewise
  logit[i,j,h] = leaky_relu(a_src[i,h] + a_dst[j,h], 0.2), -inf off-mask
  alpha = softmax over i (sources);  out[j] = sum_i alpha * x[i]  (+bias, ELU)

Device algorithm (per core, plane layout [i_part, j_free]):
  The softmax over i cancels any per-target factor, so the kernel uses
  P'' = P / exp(b_j), with  exp(lrelu(a+b) - b) = max(e^a, e^{0.2a} e^{-0.8b}):
   - a_src/a_dst come from a 16-column matmul against host-precomputed
     Wa = [W att_src | -0.8 W att_dst]  (so the device sees -0.8*b directly).
   - B'' route (all-DVE): one two-op tensor_scalar
     (e^{-0.8b}-row * e^{0.2a}-col, then max with e^a-col) and a 0/1-mask
     tensor_tensor multiply.  Exps only on [P,8]/[8,NJ] vectors.
   - R' route (ACT-heavy, R_BLOCKS i-blocks, interleaved): Relu(-0.8b - 0.8a)
     and Exp(. + a) with both a-terms as per-partition ACT bias operands,
     then the mask multiply on DVE.  Keeps VectorE and ScalarE balanced.
  [num | denom] = sum_i [x_h | 1]^T P''_h  via PE accumulation (ones column)
  out_h[j,:] = num[:,j]/denom[j]  (transpose via PE), then bias + ELU.
"""

import sys

for _p in ("/opt/trn_rl_repo",):
    if _p not in sys.path:
        sys.path.insert(0, _p)

import numpy as np

# Problem dims (fixed by the task)
B, N, D = 4, 2048, 512
H, C = 8, 64
HC = H * C
NCORES = 8
NJ = 1024        # target columns per core
P = 128
NIB = N // P     # 16 source blocks
NJB = NJ // P    # 8 target blocks
KD = D // P      # 4 contraction blocks
NEG = -1.0e9
SLOPE = 0.2
FB = 66          # per-head x-tilde stride: 64 x-cols + ones col + pad (4B align)

# stage-2 i-blocks routed through the ACT-heavy R' pipeline
# (Relu + Exp with the a-terms as ACT biases; one DVE op) instead of
# the all-DVE B'' pipeline (exp-vector outer product + max + mask).
# Spread evenly through the ib loop so ACT and DVE overlap.
R_BLOCKS = 6
R_IBS = frozenset(
    int((k + 0.5) * NIB / R_BLOCKS) for k in range(R_BLOCKS)
) if R_BLOCKS else frozenset()

_PROG = None  # cached (nc, input_names)


def _build_program():
    import concourse.bass as bass
    import concourse.mybir as mybir
    import concourse.tile as tile

    f32 = mybir.dt.float32
    bf16 = mybir.dt.bfloat16
    i32 = mybir.dt.int32
    AF = mybir.ActivationFunctionType
    OP = mybir.AluOpType

    nc = bass.Bass("TRN2", target_bir_lowering=False, debug=False)

    # node order per core: the NJ target nodes FIRST, then the rest
    # (host reorders fT columns and adj rows to match), so a_dst comes
    # from the first 8 i-blocks of the same projection pass.
    fT_d = nc.dram_tensor("fT", [D, N], f32, kind="ExternalInput").ap()
    W_d = nc.dram_tensor("W", [D, HC], f32, kind="ExternalInput").ap()
    # Wa = [W @ diag-blocks(att_src) | W @ diag-blocks(att_dst)]: [D, 2H]
    Wa_d = nc.dram_tensor("Wa", [D, 2 * H], f32, kind="ExternalInput").ap()
    adj_d = nc.dram_tensor("adj", [N, NJ], i32, kind="ExternalInput").ap()
    bias_d = nc.dram_tensor("bias", [1, HC], f32, kind="ExternalInput").ap()
    ident_d = nc.dram_tensor("ident", [P, P], f32, kind="ExternalInput").ap()
    out_d = nc.dram_tensor("out", [NJ, HC], f32, kind="ExternalOutput").ap()

    with tile.TileContext(nc) as tc:
        with (
            tc.tile_pool(name="persist", bufs=1) as pp,
            tc.tile_pool(name="dramp", bufs=1, space="DRAM") as dp,
            tc.tile_pool(name="psum_nt", bufs=3, space="PSUM") as ntp,
            tc.tile_pool(name="psum_tr", bufs=2, space="PSUM") as trp,
        ):
            # ---------------- constants ----------------
            # identity comes from the host (gpsimd affine_select would drag
            # the Pool engine's semaphore into PE waits)
            ident = pp.tile([P, P], f32, name="ident")
            nc.sync.dma_start(ident[:], ident_d[:])

            bias_b = pp.tile([P, HC], f32, name="bias_b")
            nc.sync.dma_start(bias_b[:], bias_d.to_broadcast((P, HC)))

            junk_ps = ntp.tile([1, 1], f32, name="junk_ps", tag="nt")

            # persistent cross-stage tensors.  i-blocks in R_IBS take the
            # ACT-heavy R' route (a-bias tiles); the rest the all-DVE B''
            # route (exp vectors).  Both end with a 0/1-mask multiply.
            xt = [pp.tile([P, H, FB], bf16, name=f"xt{ib}") for ib in range(NIB)]
            mask = [pp.tile([P, NJ], bf16, name=f"mk{ib}") for ib in range(NIB)]
            asrc = [
                pp.tile([P, H], f32, name=f"asrc{ib}") if ib in R_IBS else None
                for ib in range(NIB)
            ]
            a08n = [
                pp.tile([P, H], f32, name=f"a08n{ib}") if ib in R_IBS else None
                for ib in range(NIB)
            ]
            e1c = [
                pp.tile([P, H], f32, name=f"e1c{ib}") if ib not in R_IBS else None
                for ib in range(NIB)
            ]
            e2c = [
                pp.tile([P, H], f32, name=f"e2c{ib}") if ib not in R_IBS else None
                for ib in range(NIB)
            ]
            adstb = [pp.tile([P, NJ], bf16, name=f"adstb{h}") for h in range(H)]
            e1rb = [pp.tile([P, NJ], bf16, name=f"e1rb{h}") for h in range(H)]
            zt = [pp.tile([P, HC], f32, name=f"z{jt}") for jt in range(NJB)]
            adst_rows = pp.tile([H, NJ], bf16, name="adst_rows")
            e1r_rows = pp.tile([H, NJ], bf16, name="e1r_rows")

            with (
                tc.tile_pool(name="stage1", bufs=1) as s1p,
                tc.tile_pool(name="stage1st", bufs=2) as s1st,
                tc.tile_pool(name="work1", bufs=3) as wp1,
                tc.tile_pool(name="psum_x", bufs=2, space="PSUM") as xp,
                tc.tile_pool(name="psum_a", bufs=1, space="PSUM") as ap_,
            ):
                # f32 staging for weights/features, cast to bf16 for the PE
                Wt = []
                Wat = []
                fTt = []
                for kd in range(KD):
                    w_f = s1st.tile([P, HC], f32, name=f"wf_{kd}", tag="wf")
                    nc.sync.dma_start(w_f[:], W_d[kd * P:(kd + 1) * P, :])
                    w_t = s1p.tile([P, HC], bf16, name=f"w_{kd}")
                    nc.vector.tensor_copy(w_t[:], w_f[:])
                    Wt.append(w_t)
                    wa_f = s1st.tile([P, 2 * H], f32, name=f"waf_{kd}", tag="waf")
                    nc.sync.dma_start(wa_f[:], Wa_d[kd * P:(kd + 1) * P, :])
                    wa_t = s1p.tile([P, 2 * H], bf16, name=f"wa_{kd}")
                    nc.vector.tensor_copy(wa_t[:], wa_f[:])
                    Wat.append(wa_t)
                    ft_f = s1st.tile([P, N], f32, name=f"ftf_{kd}", tag="ftf")
                    nc.sync.dma_start(ft_f[:], fT_d[kd * P:(kd + 1) * P, :])
                    ft_t = s1p.tile([P, N], bf16, name=f"ft_{kd}")
                    nc.vector.tensor_copy(ft_t[:], ft_f[:])
                    fTt.append(ft_t)

                # PE instructions can carry only ONE sync wait in the walrus
                # lowering, and the PE's semaphore clock advances only via
                # its own waits. Touch the DMA-loaded ident with a tiny 1x1
                # matmul; the bf16 tiles PE reads are DVE-produced, and
                # covered DVE waits collapse in _strip_redundant_pe_waits.
                nc.tensor.matmul(
                    junk_ps[:], ident[0:1, 0:1], ident[0:1, 0:1],
                    start=True, stop=True,
                )

                # ---------------- stage 1a: a-vectors for ALL i-blocks first ----------------
                # pa matmuls are tiny (N=16); doing every block up front lets
                # the a_dst broadcast chain (and so stage-2 DVE/ACT work)
                # start ~50us earlier, overlapping the heavy projection loop.
                for ib in range(NIB):
                    pa = ap_.tile([P, 2 * H], f32, name="pa", tag="pa")
                    for kd in range(KD):
                        nc.tensor.matmul(
                            pa[:],
                            fTt[kd][:, ib * P:(ib + 1) * P],
                            Wat[kd][:],
                            start=(kd == 0),
                            stop=(kd == KD - 1),
                        )
                    # per-route a-vector tiles (PSUM readers stay on ACT:
                    # single-wait PE rule)
                    if ib in R_IBS:
                        nc.scalar.copy(asrc[ib][:], pa[:, 0:H])
                        nc.scalar.mul(a08n[ib][:], pa[:, 0:H], -0.8)
                    else:
                        nc.scalar.activation(e1c[ib][:], pa[:, 0:H], AF.Exp)
                        nc.scalar.activation(
                            e2c[ib][:], pa[:, 0:H], AF.Exp, scale=SLOPE
                        )
                    if ib < NJB:
                        # targets are the first NJB blocks: a_dst rows
                        # (pa dst columns hold -0.8*b via the host Wa scale)
                        adc = wp1.tile([P, H], f32, name="adc", tag="adc")
                        nc.scalar.copy(adc[:], pa[:, H:2 * H])
                        ptr = trp.tile([H, P], f32, name="ptr", tag="tr")
                        nc.tensor.transpose(ptr[:], adc[:], ident[:])
                        nc.vector.tensor_copy(
                            adst_rows[:, ib * P:(ib + 1) * P], ptr[:]
                        )
                    # mask: 0/1 bf16 (self-loops pre-ORed on host)
                    adj_t = wp1.tile([P, NJ], i32, name="adj_t", tag="adj")
                    nc.sync.dma_start(adj_t[:], adj_d[ib * P:(ib + 1) * P, :])
                    nc.vector.tensor_copy(mask[ib][:], adj_t[:])

                # ---------------- stage 1b: x-tilde projection ----------------
                for ib in range(NIB):
                    px = xp.tile([P, HC], f32, name="px", tag="px")
                    for kd in range(KD):
                        nc.tensor.matmul(
                            px[:],
                            fTt[kd][:, ib * P:(ib + 1) * P],
                            Wt[kd][:],
                            start=(kd == 0),
                            stop=(kd == KD - 1),
                        )
                    # x-tilde: bf16 [P, H, FB]; col 64 of each head = 1.0
                    nc.vector.memset(xt[ib][:, :, 64:65], 1.0)
                    nc.scalar.copy(
                        xt[ib][:, :, 0:64],
                        px.rearrange("p (h c) -> p h c", h=H),
                    )

                # e1r rows = exp(-0.8 b) for the B'' outer product
                nc.scalar.activation(e1r_rows[:], adst_rows[:], AF.Exp)
                # partition-broadcast via DRAM bounce (SBUF APs can't have
                # zero partition step; DRAM APs can)
                adst_dram = dp.tile([H, NJ], bf16, name="adst_dram")
                nc.sync.dma_start(adst_dram[:], adst_rows[:])
                e1r_dram = dp.tile([H, NJ], bf16, name="e1r_dram")
                nc.sync.dma_start(e1r_dram[:], e1r_rows[:])
                for h in range(H):
                    nc.sync.dma_start(
                        adstb[h][:],
                        adst_dram[h:h + 1, :].to_broadcast((P, NJ)),
                    )
                    nc.sync.dma_start(
                        e1rb[h][:],
                        e1r_dram[h:h + 1, :].to_broadcast((P, NJ)),
                    )

            # ---------------- stage 2: attention + aggregation ----------------
            with tc.tile_pool(name="work2", bufs=3) as wp:
                # PE-touch the adstb/e1rb tiles (the last DMA writes) so
                # stage-2 matmuls never need DMA-queue waits (single-wait
                # PE rule).
                for h in range(H):
                    nc.tensor.matmul(
                        junk_ps[:],
                        adstb[h][0:1, 0:1],
                        e1rb[h][0:1, 0:1],
                        start=True,
                        stop=True,
                    )
                # let ACT observe the PE tick of the touches above so
                # recycled-pool WAR waits collapse to one semaphore
                junk_sb = wp.tile([1, 1], f32, name="junk_sb", tag="jsb")
                nc.scalar.copy(junk_sb[:], junk_ps[:])
                for h in range(H):
                    nts = [
                        ntp.tile([65, 512], f32, name=f"nt{h}_{jc}", tag="nt")
                        for jc in range(2)
                    ]
                    for ib in range(NIB):
                        pe_t = wp.tile(
                            [P, NJ], bf16, name="pe_t", tag="pe", bufs=16
                        )
                        # The softmax over sources i cancels any per-target
                        # factor, so use P'' = P / exp(b_j):
                        #   exp(lrelu(a+b) - b) = max(e^a, e^{0.2a} e^{-0.8b})
                        if ib in R_IBS:
                            # ACT-heavy R': exp(max(...)) = exp(a + relu(
                            # -0.8 b - 0.8 a)); both a-terms ride ACT biases,
                            # then the 0/1 mask multiply on DVE.
                            r = wp.tile([P, NJ], bf16, name="r", tag="s")
                            nc.scalar.activation(
                                r[:], adstb[h][:], AF.Relu,
                                bias=a08n[ib][:, h:h + 1], scale=1.0,
                            )
                            er = wp.tile([P, NJ], bf16, name="er", tag="lr")
                            nc.scalar.activation(
                                er[:], r[:], AF.Exp,
                                bias=asrc[ib][:, h:h + 1], scale=1.0,
                            )
                            nc.vector.tensor_mul(pe_t[:], er[:], mask[ib][:])
                        else:
                            # all-DVE B'': outer product of exp vectors and
                            # the max with e^a fused in one two-op
                            # tensor_scalar, then the 0/1 mask multiply.
                            u = wp.tile([P, NJ], bf16, name="u", tag="s2")
                            nc.vector.tensor_scalar(
                                out=u[:],
                                in0=e1rb[h][:],
                                scalar1=e2c[ib][:, h:h + 1],
                                scalar2=e1c[ib][:, h:h + 1],
                                op0=OP.mult,
                                op1=OP.max,
                            )
                            nc.vector.tensor_mul(pe_t[:], u[:], mask[ib][:])
                        for jc in range(2):
                            nc.tensor.matmul(
                                nts[jc][:],
                                xt[ib][:, h, 0:65],
                                pe_t[:, jc * 512:(jc + 1) * 512],
                                start=(ib == 0),
                                stop=(ib == NIB - 1),
                            )
                    # post: transpose back to [j, c], divide by denominator
                    for jc in range(2):
                        nt_sb = wp.tile([65, 512], f32, name="nt_sb", tag="ntsb")
                        nc.scalar.copy(nt_sb[:], nts[jc][:])
                        for jq in range(4):
                            jt = jc * 4 + jq
                            ptq = trp.tile([P, 65], f32, name="ptq", tag="tr")
                            nc.tensor.transpose(
                                ptq[:],
                                nt_sb[:, jq * P:(jq + 1) * P],
                                ident[0:65, 0:65],
                            )
                            # PSUM bank readers must stay ACT-only so the
                            # next PE transpose reusing it has one wait.
                            rec = wp.tile([P, 1], f32, name="rec", tag="rec")
                            nc.vector.reciprocal(rec[:], ptq[:, 64:65])
                            nc.scalar.activation(
                                zt[jt][:, h * C:(h + 1) * C],
                                ptq[:, 0:64],
                                AF.Copy,
                                scale=rec[:],
                            )

                # ---------------- stage 3: bias + ELU + store ----------------
                for jt in range(NJB):
                    zb = wp.tile([P, HC], f32, name="zb", tag="zb")
                    nc.vector.tensor_add(zb[:], zt[jt][:], bias_b[:])
                    ee = wp.tile([P, HC], f32, name="ee", tag="ee")
                    nc.scalar.activation(ee[:], zb[:], AF.Exp)
                    # elu(z) = relu(z) + min(exp(z) - 1, 0)
                    em = wp.tile([P, HC], f32, name="em", tag="em")
                    nc.vector.tensor_scalar(
                        out=em[:],
                        in0=ee[:],
                        scalar1=-1.0,
                        scalar2=0.0,
                        op0=OP.add,
                        op1=OP.min,
                    )
                    of = wp.tile([P, HC], f32, name="of", tag="of")
                    nc.vector.scalar_tensor_tensor(
                        out=of[:],
                        in0=zb[:],
                        scalar=0.0,
                        in1=em[:],
                        op0=OP.max,
                        op1=OP.add,
                    )
                    nc.sync.dma_start(out_d[jt * P:(jt + 1) * P, :], of[:])

    _strip_redundant_pe_waits(nc)
    _split_excess_waits(nc)
    return nc


# empirical per-engine sync-wait budgets in the walrus CoreV3 lowering
_WAIT_BUDGET = {
    "EngineType.PE": 1,
    "EngineType.Activation": 1,
    "EngineType.DVE": 1,
    "EngineType.Pool": 1,
    "EngineType.SP": 1,
}


def _split_excess_waits(nc):
    """Instructions whose on_wait exceeds the engine's wait budget get the
    excess waits moved onto NoOp instructions inserted just before them in
    the same (in-order) engine queue."""
    import concourse.mybir as mybir

    fn = nc.m.functions[0]
    n = 0
    for blk in fn.blocks:
        insts = blk.instructions
        k = 0
        while k < len(insts):
            i = insts[k]
            eng = str(getattr(i, "engine", ""))
            si = getattr(i, "sync_info", None)
            budget = _WAIT_BUDGET.get(eng)
            if type(i).__name__ == "InstTensorScalarPtr":
                # S2S2D2_STT lowering (CoreV2 path) allows only one wait
                budget = 1
            if si is None or budget is None or len(si.on_wait) <= budget:
                k += 1
                continue
            ws = list(si.on_wait)
            excess, keep = ws[: len(ws) - budget], ws[len(ws) - budget:]
            for w in excess:
                nop = mybir.InstNoOp(name=f"I-wsplit{n}", ins=[], outs=[])
                n += 1
                nop.engine = i.engine
                nop.sync_info = type(si)(on_wait=[w], on_update=[])
                insts.insert(k, nop)
                k += 1
            si.on_wait = keep
            i.sync_info = si
            k += 1


def _strip_redundant_pe_waits(nc):
    """walrus allows only ONE sync wait per PE instruction. Tile emits
    [bank-reader-sem, PE-self-sem] pairs on PSUM slot reuse even though the
    reader wait transitively implies the PE WAW wait (the reader itself
    waited for the PE chain). Compute, per instruction in scheduled order,
    the PE tick each semaphore value transitively certifies, and drop PE
    self-waits that are covered by a co-occurring wait."""
    fn = nc.m.functions[0]
    flat = [i for blk in fn.blocks for i in blk.instructions]

    def _merge(dst, src):
        for k, v in src.items():
            if dst.get(k, 0) < v:
                dst[k] = v

    # engine -> its own completion semaphore (each engine executes its
    # instruction stream strictly in order, so waits on the engine's own
    # sem are always satisfied at dispatch and can be dropped)
    self_sem = {}
    for i in flat:
        si = getattr(i, "sync_info", None)
        eng = str(getattr(i, "engine", ""))
        if si is None or type(i).__name__ in ("InstNop", "InstDrain"):
            continue
        if eng not in self_sem and si.on_update:
            nm = si.on_update[0].ant_name
            if not nm.startswith(("DMAHW", "DMASW", "barrier")):
                self_sem[eng] = nm

    obs = {}        # engine -> observed vector clock {sem: tick}
    events = {}     # (sem, value) -> vector clock certified when sem hit value
    sem_val = {}
    for i in flat:
        eng = str(getattr(i, "engine", ""))
        si = getattr(i, "sync_info", None)
        if si is None:
            continue
        o = obs.setdefault(eng, {})
        for w in si.on_wait:
            if w.wait_value is None:
                continue
            if o.get(w.ant_name, 0) < w.wait_value:
                o[w.ant_name] = w.wait_value
            _merge(o, events.get((w.ant_name, w.wait_value), {}))
        if any(w.ant_name == self_sem.get(eng) for w in si.on_wait):
            si.on_wait = [
                w for w in si.on_wait if w.ant_name != self_sem.get(eng)
            ]
            i.sync_info = si
        if len(si.on_wait) > 1:
            ws = [w for w in si.on_wait]
            certs = []
            for w in ws:
                c = dict(events.get((w.ant_name, w.wait_value), {})) \
                    if w.wait_value is not None else {}
                if w.wait_value is not None:
                    c[w.ant_name] = max(c.get(w.ant_name, 0), w.wait_value)
                certs.append(c)
            # greedily keep waits not covered by the union of kept certs
            order = sorted(range(len(ws)), key=lambda j: -len(certs[j]))
            kept, covered = [], {}
            for j in order:
                w = ws[j]
                if (
                    w.wait_value is not None
                    and covered.get(w.ant_name, 0) >= w.wait_value
                ):
                    continue
                kept.append(j)
                _merge(covered, certs[j])
            if len(kept) < len(ws):
                si.on_wait = [ws[j] for j in sorted(kept)]
                i.sync_info = si
        for u in si.on_update:
            if u.update_value is None:
                continue
            v1 = sem_val.get(u.ant_name, 0) + u.update_value
            sem_val[u.ant_name] = v1
            cert = dict(o)
            cert[u.ant_name] = max(cert.get(u.ant_name, 0), v1)
            for vv in range(v1 - u.update_value + 1, v1 + 1):
                events[(u.ant_name, vv)] = cert
            if o.get(u.ant_name, 0) < v1:
                o[u.ant_name] = v1


def _get_program():
    global _PROG
    if _PROG is None:
        _PROG = _build_program()
    return _PROG


def _make_in_maps(features_batch, adj_mats_batch, W, att_src, att_dst, bias):
    f = np.asarray(features_batch, dtype=np.float32)
    adj = np.asarray(adj_mats_batch, dtype=np.int32)
    Wn = np.ascontiguousarray(np.asarray(W, dtype=np.float32))
    asv = np.asarray(att_src, dtype=np.float32).reshape(H, C)
    adv = np.asarray(att_dst, dtype=np.float32).reshape(H, C)
    bv = np.ascontiguousarray(np.asarray(bias, dtype=np.float32).reshape(1, HC))

    # Wa[d, h] = sum_c W[d, h*C+c] * att_src[h, c]  (and att_dst);
    # the dst half is prescaled by -0.8 so the device gets -0.8*b rows
    # directly (used raw by the R' route and exp'd by the B'' route).
    W3 = Wn.reshape(D, H, C)
    wa = np.empty((D, 2 * H), dtype=np.float32)
    wa[:, 0:H] = np.einsum("dhc,hc->dh", W3, asv)
    wa[:, H:2 * H] = np.einsum("dhc,hc->dh", W3, adv) * -0.8
    wa = np.ascontiguousarray(wa)

    ident = np.eye(P, dtype=np.float32)
    in_maps = []
    jdx = np.arange(NJ)
    for c in range(NCORES):
        b, half = divmod(c, 2)
        j0 = half * NJ
        # node order: the NJ target nodes first, then the other half
        order = np.r_[j0:j0 + NJ, (NJ - j0):(2 * NJ - j0)]
        adjs = np.ascontiguousarray(adj[b][order][:, j0:j0 + NJ])
        adjs[jdx, jdx] = 1  # self-loops always present
        fT = np.ascontiguousarray(f[b][order].T)
        in_maps.append(
            {
                "fT": fT,
                "W": Wn,
                "Wa": wa,
                "adj": adjs,
                "bias": bv,
                "ident": ident,
            }
        )
    return in_maps


_RUNNER = None  # cached (jitted_fn, in_names, out_names, n_params, zero_outs)


def _get_runner():
    """Build a jitted shard_map runner for the bass program (mirrors
    concourse.bass2jax.run_bass_via_pjrt but without output donation, so
    device-resident inputs can be reused across timed iterations)."""
    global _RUNNER
    if _RUNNER is not None:
        return _RUNNER
    import jax
    import concourse.mybir as mybir
    from concourse import bass2jax
    from jax.sharding import Mesh, PartitionSpec
    from jax.experimental.shard_map import shard_map

    bass2jax.install_neuronx_cc_hook()
    nc = _get_program()

    partition_name = (
        nc.partition_id_tensor.name if nc.partition_id_tensor else None
    )
    in_names, out_names, out_avals, zero_outs = [], [], [], []
    for alloc in nc.m.functions[0].allocations:
        if not isinstance(alloc, mybir.MemoryLocationSet):
            continue
        name = alloc.memorylocations[0].name
        if alloc.kind == "ExternalInput":
            if name != partition_name:
                in_names.append(name)
        elif alloc.kind == "ExternalOutput":
            shape = tuple(alloc.tensor_shape)
            dtype = mybir.dt.np(alloc.dtype)
            out_names.append(name)
            out_avals.append(jax.core.ShapedArray(shape, dtype))
            zero_outs.append(np.zeros(shape, dtype))
    n_params = len(in_names)
    all_names = in_names + out_names
    if partition_name is not None:
        all_names = all_names + [partition_name]

    def _body(*args):
        operands = list(args)
        if partition_name is not None:
            operands.append(bass2jax.partition_id_tensor())
        outs = bass2jax._bass_exec_p.bind(
            *operands,
            out_avals=tuple(out_avals),
            in_names=tuple(all_names),
            out_names=tuple(out_names),
            lowering_input_output_aliases=(),
            sim_require_finite=True,
            sim_require_nnan=True,
            nc=nc,
        )
        return tuple(outs)

    devices = jax.devices()[:NCORES]
    mesh = Mesh(np.asarray(devices), ("core",))
    n_args = n_params + len(out_names)
    jitted = jax.jit(
        shard_map(
            _body,
            mesh=mesh,
            in_specs=(PartitionSpec("core"),) * n_args,
            out_specs=(PartitionSpec("core"),) * len(out_names),
            check_rep=False,
        ),
        keep_unused=True,
    )
    _RUNNER = (jitted, in_names, out_names, n_params, zero_outs)
    return _RUNNER


def _sharded_device_put(concat_in):
    import jax
    from jax.sharding import Mesh, PartitionSpec, NamedSharding

    devices = jax.devices()[:NCORES]
    mesh = Mesh(np.asarray(devices), ("core",))
    sh = NamedSharding(mesh, PartitionSpec("core"))
    return jax.device_put(concat_in, sh)


def make_device_runner(inputs_dict):
    """Build (run_once, out_check): one warm 8-core inference on
    device-resident pre-sharded inputs, and an output assembler."""
    import jax

    in_maps = _make_in_maps(**inputs_dict)
    jitted, in_names, out_names, n_params, zero_outs = _get_runner()
    concat_in = [
        np.concatenate([m[name] for m in in_maps], axis=0) for name in in_names
    ] + [
        np.concatenate([z] * NCORES, axis=0) for z in zero_outs
    ]
    dev_in = _sharded_device_put(concat_in)

    def run_once():
        outs = jitted(*dev_in)
        jax.block_until_ready(outs)
        return outs

    def out_check(outs):
        np_outs = [np.asarray(o) for o in outs]
        results = [
            {
                name: np_outs[i][c * NJ:(c + 1) * NJ]
                for i, name in enumerate(out_names)
            }
            for c in range(NCORES)
        ]
        return _assemble(results)

    return run_once, out_check


def _run(in_maps, time_iters=0):
    """Execute on 8 cores. Returns (results_list, min_wall_ns or None)."""
    import jax

    jitted, in_names, out_names, n_params, zero_outs = _get_runner()
    concat_in = [
        np.concatenate([m[name] for m in in_maps], axis=0) for name in in_names
    ] + [
        np.concatenate([z] * NCORES, axis=0) for z in zero_outs
    ]
    dev_in = _sharded_device_put(concat_in)
    outs = jitted(*dev_in)
    jax.block_until_ready(outs)

    best_ns = None
    if time_iters > 0:
        import time as _time

        for _ in range(time_iters):
            t0 = _time.perf_counter()
            outs2 = jitted(*dev_in)
            jax.block_until_ready(outs2)
            dt = (_time.perf_counter() - t0) * 1e9
            best_ns = dt if best_ns is None else min(best_ns, dt)
        outs = outs2

    results = []
    np_outs = [np.asarray(o) for o in outs]
    per_core = NJ  # axis-0 length of each core's "out"
    for c in range(NCORES):
        results.append(
            {
                name: np_outs[i][c * per_core:(c + 1) * per_core]
                for i, name in enumerate(out_names)
            }
        )
    return results, best_ns


def _assemble(results):
    out = np.empty((B, N, HC), dtype=np.float32)
    for c in range(NCORES):
        b, half = divmod(c, 2)
        j0 = half * NJ
        out[b, j0:j0 + NJ, :] = results[c]["out"]
    return out


def kernel(features_batch, adj_mats_batch, W, att_src, att_dst, bias):
    in_maps = _make_in_maps(
        features_batch, adj_mats_batch, W, att_src, att_dst, bias
    )
    results, _ = _run(in_maps)
    return _assemble(results)


def run_profiled(features_batch, adj_mats_batch, W, att_src, att_dst, bias,
                 time_iters=10):
    """Like kernel() but also times warm executions; returns (out, min_ns)."""
    in_maps = _make_in_maps(
        features_batch, adj_mats_batch, W, att_src, att_dst, bias
    )
    results, best_ns = _run(in_maps, time_iters=time_iters)
    return _assemble(results), best_ns

